# revision 2
# baseline (speedup 1.0000x reference)
"""Trainium2 Bass kernel for a DoReFa-quantized DenseNet basic block.

Computes, for x:[128,256,32,32] f32:
  bn   = x * inv + (beta - mean*inv)          (inference BatchNorm)
  aq   = round(15 * clip(bn, 0, 1)) / 15      (4-bit activation quant, RNE)
  wq   = 2*round(15*wn)/15 - 1                (4-bit weight quant, host-side)
  conv = conv2d(aq, wq, 3x3, pad 1)
  out  = concat([x, conv], axis=1)            -> [128, 268, 32, 32]

Strategy: data-parallel over batch across 8 NeuronCores (16 images each).
The quantized activations are exact small integers 0..15 and quantized
weights are exact odd integers -15..15, so the conv runs on the PE array in
bf16 with *exact* integer arithmetic (fp32 PSUM accumulation), scaled by
1/225 on the way out.  The 3x3 conv is 18 PSUM-accumulated matmuls per
512-pixel chunk: one [128C x 12G] weight tap against a W-padded activation
tile with shifted access patterns (9 taps x 2 C-halves).
"""

from contextlib import ExitStack

import numpy as np
import ml_dtypes

import jax
import concourse.bass as bass
import concourse.tile as tile
from concourse import bacc, mybir
from concourse.bass2jax import _bass_exec_p, install_neuronx_cc_hook, partition_id_tensor
from jax.experimental.shard_map import shard_map
from jax.sharding import Mesh, PartitionSpec

N_CORES = 8
B, C, H, W = 128, 256, 32, 32
G = 12            # growthRate (conv output channels)
B_LOC = B // N_CORES
HW = H * W
BN_EPS = 1e-5
MAGIC = 8388608.0  # 2**23: adding then subtracting rounds fp32 to nearest int (RNE)

_CACHE: dict = {}


def _build_nc():
    f32 = mybir.dt.float32
    bf16 = mybir.dt.bfloat16
    nc = bacc.Bacc("TRN2", target_bir_lowering=False, debug=False, num_devices=N_CORES)

    x = nc.dram_tensor("x", [B_LOC, C, HW], f32, kind="ExternalInput")
    bn_scale = nc.dram_tensor("bn_scale", [128, 2], f32, kind="ExternalInput")
    bn_bias = nc.dram_tensor("bn_bias", [128, 2], f32, kind="ExternalInput")
    wq = nc.dram_tensor("wq", [2, 3, 3, 128, G], bf16, kind="ExternalInput")
    out = nc.dram_tensor("out", [B_LOC, C + G, HW], f32, kind="ExternalOutput")

    with ExitStack() as ctx:
        tc = ctx.enter_context(tile.TileContext(nc))
        singles = ctx.enter_context(tc.tile_pool(name="singles", bufs=1))
        xin = ctx.enter_context(tc.tile_pool(name="xin", bufs=3))
        tmp = ctx.enter_context(tc.tile_pool(name="tmp", bufs=2))
        apad = ctx.enter_context(tc.tile_pool(name="apad", bufs=2))
        pspool = ctx.enter_context(tc.tile_pool(name="ps", bufs=4, space="PSUM"))
        cout = ctx.enter_context(tc.tile_pool(name="cout", bufs=2))

        w_tile = singles.tile([128, 2, 3, 3, G], bf16)
        nc.sync.dma_start(out=w_tile[:], in_=wq[:].rearrange("g h w p o -> p g h w o"))
        bns = singles.tile([128, 2], f32)
        nc.sync.dma_start(out=bns[:], in_=bn_scale[:])
        bnb = singles.tile([128, 2], f32)
        nc.sync.dma_start(out=bnb[:], in_=bn_bias[:])

        for img in range(B_LOC):
            x_tile = xin.tile([128, 2, HW], f32)
            nc.sync.dma_start(
                out=x_tile[:], in_=x[img].rearrange("(g p) m -> p g m", p=128)
            )
            # bn = relu(x*inv + shift)  (per-channel scale/bias, lower clip)
            t_tile = tmp.tile([128, 2, HW], f32, tag="t")
            for g in range(2):
                nc.scalar.activation(
                    out=t_tile[:, g],
                    in_=x_tile[:, g],
                    func=mybir.ActivationFunctionType.Relu,
                    bias=bnb[:, g : g + 1],
                    scale=bns[:, g : g + 1],
                )
            # u = 15*min(bn,1) + 2^23   (upper clip, scale, begin RNE round)
            u_tile = tmp.tile([128, 2, HW], f32, tag="u")
            nc.vector.tensor_scalar(
                u_tile[:],
                t_tile[:],
                1.0,
                15.0,
                mybir.AluOpType.min,
                mybir.AluOpType.mult,
            )
            # a = (u + 2^23) - 2^23 -> integer 0..15, cast bf16, write into
            # W-padded layout [128, g, h, 1+w] (cols 0 and 33 are zero pad)
            a_tile = apad.tile([128, 2, H, W + 2], bf16)
            nc.vector.memset(a_tile[:, :, :, 0 : W + 2 : W + 1], 0.0)
            nc.vector.tensor_scalar(
                a_tile[:, :, :, 1 : W + 1],
                u_tile[:].rearrange("p g (h w) -> p g h w", w=W),
                MAGIC,
                MAGIC,
                mybir.AluOpType.add,
                mybir.AluOpType.subtract,
            )
            # 3x3 conv via 18 PSUM-accumulated matmuls per 512-pixel chunk
            co = cout.tile([G, HW], f32)
            for ch in range(2):
                h0 = ch * 16
                ps = pspool.tile([G, 512], f32)
                taps = [
                    (dh, dw, g) for dh in (0, -1, 1) for dw in (-1, 0, 1) for g in range(2)
                ]
                for i, (dh, dw, g) in enumerate(taps):
                    hlo = max(h0, -dh)
                    hhi = min(h0 + 16, H - dh)
                    rhs = a_tile[:, g, hlo + dh : hhi + dh, 1 + dw : W + 1 + dw]
                    nc.tensor.matmul(
                        ps[:, (hlo - h0) * W : (hhi - h0) * W],
                        w_tile[:, g, dh + 1, dw + 1, :],
                        rhs,
                        start=(i == 0),
                        stop=(i == len(taps) - 1),
                        skip_group_check=True,
                    )
                nc.scalar.activation(
                    out=co[:, ch * 512 : (ch + 1) * 512],
                    in_=ps[:],
                    func=mybir.ActivationFunctionType.Copy,
                    scale=1.0 / 225.0,
                )
            nc.sync.dma_start(
                out=out[img, 0:C].rearrange("(g p) m -> p g m", p=128), in_=x_tile[:]
            )
            nc.sync.dma_start(out=out[img, C : C + G], in_=co[:])
    nc.compile()
    return nc


def _get_runner():
    """Build (once) a jitted 8-core sharded executor for the bass kernel.

    Mirrors bass2jax.run_bass_via_pjrt's multi-core branch, but caches the
    jitted callable so repeated kernel() calls don't re-trace/re-compile.
    No donation: the kernel writes every output element.
    """
    if "runner" in _CACHE:
        return _CACHE["runner"]

    install_neuronx_cc_hook()
    nc = _build_nc()
    partition_name = nc.partition_id_tensor.name if nc.partition_id_tensor else None

    in_names: list[str] = []
    out_names: list[str] = []
    out_avals: list[jax.core.ShapedArray] = []
    zero_outs: list[np.ndarray] = []
    for alloc in nc.m.functions[0].allocations:
        if not isinstance(alloc, mybir.MemoryLocationSet):
            continue
        name = alloc.memorylocations[0].name
        if alloc.kind == "ExternalInput":
            if name != partition_name:
                in_names.append(name)
        elif alloc.kind == "ExternalOutput":
            shape = tuple(alloc.tensor_shape)
            dtype = mybir.dt.np(alloc.dtype)
            out_names.append(name)
            out_avals.append(jax.core.ShapedArray(shape, dtype))
            zero_outs.append(np.zeros(shape, dtype))
    n_params = len(in_names)
    all_in_names = in_names + out_names
    if partition_name is not None:
        all_in_names = all_in_names + [partition_name]

    def _body(*args):
        operands = list(args)
        if partition_name is not None:
            operands.append(partition_id_tensor())
        outs = _bass_exec_p.bind(
            *operands,
            out_avals=tuple(out_avals),
            in_names=tuple(all_in_names),
            out_names=tuple(out_names),
            lowering_input_output_aliases=(),
            sim_require_finite=True,
            sim_require_nnan=True,
            nc=nc,
        )
        return tuple(outs)

    devices = jax.devices()[:N_CORES]
    mesh = Mesh(np.asarray(devices), ("core",))
    n_outs = len(out_names)
    sharded = jax.jit(
        shard_map(
            _body,
            mesh=mesh,
            in_specs=(PartitionSpec("core"),) * (n_params + n_outs),
            out_specs=(PartitionSpec("core"),) * n_outs,
            check_rep=False,
        ),
        keep_unused=True,
    )
    runner = (sharded, in_names, out_names, zero_outs)
    _CACHE["runner"] = runner
    return runner


def _host_prep(x, gamma, beta, mean, var, weight):
    """Host-side prep: fold BN params, quantize the tiny conv weight."""
    inv = (gamma / np.sqrt(var + BN_EPS)).astype(np.float32)
    shift = (beta - mean * inv).astype(np.float32)
    bn_scale = inv.reshape(2, 128).T.copy()  # [p, g] with c = g*128 + p
    bn_bias = shift.reshape(2, 128).T.copy()

    # DoReFa weight quant (forward value): wq = 2*round(15*wn)/15 - 1,
    # wn = tanh(w)/(2*max|tanh(w)|) + 0.5.  Stored as integer 15*wq.
    t = np.tanh(weight.astype(np.float32))
    wn = t / (2.0 * np.abs(t).max()) + np.float32(0.5)
    q15 = np.round(wn * np.float32(15.0))
    w_int = (2.0 * q15 - 15.0).astype(np.float32)  # [G, C, 3, 3], odd ints
    # lhsT layout [g, kh, kw, c_in_half, oc]
    wq_l = np.ascontiguousarray(
        w_int.reshape(G, 2, 128, 3, 3).transpose(1, 3, 4, 2, 0)
    ).astype(ml_dtypes.bfloat16)
    return bn_scale, bn_bias, wq_l


def kernel(x, gamma, beta, mean, var, weight):
    x = np.asarray(x, dtype=np.float32)
    bn_scale, bn_bias, wq_l = _host_prep(
        x,
        np.asarray(gamma, np.float32),
        np.asarray(beta, np.float32),
        np.asarray(mean, np.float32),
        np.asarray(var, np.float32),
        np.asarray(weight, np.float32),
    )
    sharded, in_names, out_names, zero_outs = _get_runner()

    x3 = x.reshape(B, C, HW)  # batch-sharded: core c gets rows [16c, 16c+16)
    per_input = {
        "x": x3,
        "bn_scale": np.concatenate([bn_scale] * N_CORES, axis=0),
        "bn_bias": np.concatenate([bn_bias] * N_CORES, axis=0),
        "wq": np.concatenate([wq_l] * N_CORES, axis=0),
    }
    concat_in = [per_input[name] for name in in_names]
    concat_zeros = [
        np.zeros((N_CORES * z.shape[0], *z.shape[1:]), z.dtype) for z in zero_outs
    ]
    out_arrs = sharded(*concat_in, *concat_zeros)
    out = np.asarray(out_arrs[out_names.index("out")])  # [B, C+G, HW]
    return out.reshape(B, C + G, H, W)
